# revision 34
# baseline (speedup 1.0000x reference)
"""Trainium2 Bass kernel for a dense transformer block (LN -> 16-head causal
attention -> residual -> LN -> FFN -> residual) on x:(2, 2048, 1024) fp32.

Sharding: 8 cores, zero collectives. Core c handles batch b=c//4, query chunk
a=c%4 (512 contiguous tokens). Every core recomputes full-sequence K/V for its
batch (replicated compute instead of collectives: the cost model prices an
AllGather of K/V at ~226us, far above the ~28us of redundant projection work).

Key tricks (all data-driven so the compiled program is identical across cores):
- Token permutation: each core's K/V token order puts its OWN 512 query tokens
  first. The 4 leading 128-token key chunks are then exactly the "diagonal"
  causal blocks for every core, so the additive causal mask is a core-invariant
  constant applied to a fixed set of psum blocks (folded into the score matmul
  via an identity-lhsT accumulate, not a vector op).
- V gating: chunks that a core's queries may never attend (future tokens) are
  zeroed at V-evacuation time via a per-core gate column, and the softmax
  denominator "ones" column is gated the same way. exp() of those scores still
  runs (uniform program) but contributes exactly 0.
- fp8 (e4m3) DoubleRow matmuls for Q/K/V/out projections, AV, and both FFN
  layers: weights are pre-scaled by 32 on the host to sit in e4m3's sweet spot;
  the inverse scales fold into existing psum-evacuation ops. Scores stay bf16
  (contraction is 64-deep; DoubleRow needs 128-pairs). LayerNorm scale/shift
  (g, be) fold into the weights/biases on the host.
- LayerNorm statistics via ones-vector matmuls (partition reduction); the four
  512-token chunks' stats land on psum (partition 0/64 x free half) so the
  scalar math runs once over (65, 2, 512) views.
- softmax exp runs on the Activation engine over PAIRS of score psum banks
  (one instruction per 2 key-chunks), writing fp8 pairs consumed directly by
  DoubleRow AV matmuls.
Engine balance: psum evacuations are spread over DVE/Act/Pool (K-evac + square
on Act, normalize-subtract on Pool, rest on DVE).
"""

import numpy as np
import ml_dtypes

import concourse.bass as bass
import concourse.tile as tile
from concourse import bacc, mybir
from concourse import bass_utils
from concourse.bass import ts

P = 128
B, T, C = 2, 2048, 1024
H, D = 16, 64
FF = 4 * C
CC = C // P          # 8 feature chunks
TQ = 512             # queries per core
NSCH = T // P        # 16 key chunks
NDIAG = 4            # leading chunks = the core's own (diagonal) tokens
EPS = 1e-5
NEG = -30000.0
WS = 32.0            # host-side fp8 weight scale
OTS = 64.0           # attention-output fp8 scale

bf16 = ml_dtypes.bfloat16
f8 = ml_dtypes.float8_e4m3fn

f32 = mybir.dt.float32
bf = mybir.dt.bfloat16
fp8 = mybir.dt.float8e4
AF = mybir.ActivationFunctionType
ALU = mybir.AluOpType
DR = mybir.MatmulPerfMode.DoubleRow


def _ln_stats(nc, stp, big1, chunked, xs, eps1, ones1):
    """LN stats of one 512-token chunk xs (P, CC, 512) bf16. Sum lands on
    psum partition 0, sum-of-squares on partition 64 (one bank). Returns
    (a, ma) (1, 512) bf16 tiles: rsqrt(var) and mean*rsqrt(var)."""
    st = stp.tile([65, 512], f32, tag="st")
    ps = st[0:1, :]
    pq = st[64:65, :]
    for cc in range(CC):
        nc.tensor.matmul(ps, lhsT=ones1, rhs=xs[:, cc, :],
                         start=(cc == 0), stop=(cc == CC - 1))
    for cc in range(CC):
        sq = chunked.tile([P, 512], bf, tag="st_sqt")
        nc.scalar.activation(sq, xs[:, cc, :], AF.Square)
        nc.tensor.matmul(pq, lhsT=ones1, rhs=sq,
                         start=(cc == 0), stop=(cc == CC - 1))
    nc.vector.tensor_scalar_mul(st, st, 1.0 / C)   # m, E[x^2] (garbage rows ok)
    m, v = ps, pq
    msq = big1.tile([1, 512], f32, tag="st_tmp")
    nc.scalar.activation(msq, m, AF.Square)
    nc.vector.tensor_tensor(v, v, msq, ALU.subtract)      # v := var
    sd = big1.tile([1, 512], f32, tag="st_tmp2")
    nc.scalar.activation(sd, v, AF.Sqrt, bias=eps1[0:1, :])
    a = big1.tile([1, 512], bf, tag="st_a")
    with nc.allow_low_precision(reason="LN scale in bf16 is within budget"):
        nc.vector.reciprocal(a, sd)
    ma = big1.tile([1, 512], bf, tag="st_ma")
    nc.vector.tensor_mul(ma, m, a)
    return a, ma


def _ln_norm(nc, chunked, xs, out8c, a, ma, on_pool=3):
    """out8c fp8 = (xs - m) * a. mult on DVE; subtract split DVE/Pool
    (on_pool = out of 8 cc on Pool)."""
    a_bc = chunked.tile([P, 512], bf, tag="nm_abc")
    nc.gpsimd.partition_broadcast(a_bc, a)
    ma_bc = chunked.tile([P, 512], bf, tag="nm_mabc")
    nc.gpsimd.partition_broadcast(ma_bc, ma)
    for cc in range(CC):
        t = chunked.tile([P, 512], bf, tag="nm_t")
        nc.vector.tensor_mul(t, xs[:, cc, :], a_bc)
        if cc % 2 == 0 and on_pool:
            nc.gpsimd.tensor_tensor(out8c[:, cc, :], t, ma_bc, ALU.subtract)
        else:
            nc.vector.tensor_tensor(out8c[:, cc, :], t, ma_bc, ALU.subtract)


def _body(nc, tc, aps):
    (xkvT, xqT, maskT, gateT, vonesT, eyeT, wq, wk, wv, wo, w1, w2,
     b1_t, b2_t, outT) = aps

    import contextlib
    ctx = contextlib.ExitStack()
    with ctx:
        consts = ctx.enter_context(tc.tile_pool(name="consts", bufs=1))
        small = ctx.enter_context(tc.tile_pool(name="small", bufs=2))

        ones1 = consts.tile([P, 1], bf)
        nc.vector.memset(ones1, 1.0)
        eps1 = consts.tile([P, 1], f32)
        nc.vector.memset(eps1, EPS)

        def load(pool, ap_dram, shape, dtype=f32, tag=None):
            t = pool.tile(list(shape), dtype, tag=tag or ap_dram.name)
            nc.sync.dma_start(t, ap_dram)
            return t

        eye_s = load(consts, eyeT, (P, P), bf)
        gate_s = load(consts, gateT, (P, NSCH))
        b1_s = load(consts, b1_t, (P, 32))
        b2_s = load(consts, b2_t, (P, CC))
        mask_s = load(consts, maskT, (P, NDIAG, TQ), bf)
        vones_s = load(consts, vonesT, (P, NSCH, H), fp8)

        opool = ctx.enter_context(tc.tile_pool(name="opool", bufs=1))
        pf = ctx.enter_context(tc.tile_pool(name="p_late", bufs=1))

        # ---- long-lived activation tiles
        kvq_ctx = contextlib.ExitStack()
        kvq = kvq_ctx.enter_context(tc.tile_pool(name="kvq", bufs=1))
        KT = kvq.tile([P, 8, T], bf)
        Vr = kvq.tile([P, NSCH, H, 65], fp8)
        QT = kvq.tile([P, 8, TQ], bf)

        # ---- phase A: LN1 + Q/K/V projections, pipelined per 512-chunk ----
        with tc.tile_pool(name="hpool", bufs=1) as hpool, \
             tc.tile_pool(name="p_xc", bufs=2) as pxc, \
             tc.tile_pool(name="p_ln1", bufs=2) as p1, \
             tc.tile_pool(name="p_ln1t", bufs=3) as p1t, \
             tc.tile_pool(name="p_w", bufs=1) as pw, \
             tc.tile_pool(name="pp_a", bufs=4, space="PSUM") as pp_a, \
             tc.tile_pool(name="st_a", bufs=2, space="PSUM") as st_a:
            xc0 = pxc.tile([P, CC, 512], bf, tag="xkv")
            nc.sync.dma_start(xc0, xkvT[:, :, ts(0, 512)])
            wk_s = load(pw, wk, (P, CC, 8, P), fp8)
            wv_s = load(pw, wv, (P, CC, C), fp8)
            wq_s = load(pw, wq, (P, CC, 8, P), fp8)
            h8 = hpool.tile([P, CC, T], fp8)
            # gated ones column for the softmax denominator
            nc.vector.tensor_copy(
                Vr[:, :, :, 64:65].rearrange("p a h e -> p a (h e)"), vones_s)

            def chunk_projs(ch, h8c):
                # K rows for this chunk's 512 tokens (all 8 feature pairs).
                # KT carries the x32 weight scale (folded into Q's evac).
                for pair in range(8):
                    psum = pp_a.tile([P, 512], f32, tag="mm")
                    for cc2 in range(4):
                        nc.tensor.matmul(
                            psum,
                            lhsT=wk_s[:, 2 * cc2:2 * cc2 + 2, pair, :],
                            rhs=h8c[:, 2 * cc2:2 * cc2 + 2, :],
                            start=(cc2 == 0), stop=(cc2 == 3), perf_mode=DR)
                    nc.scalar.copy(KT[:, pair, ts(ch, 512)], psum)

                # V rows (4 token subchunks x 2 feature halves); gated evac
                # (gate carries the 1/32 rescale), fp8 out, split DVE/Act.
                for j in range(4):
                    st = 4 * ch + j
                    for half in range(2):
                        psum = pp_a.tile([P, 512], f32, tag="mm")
                        for cc2 in range(4):
                            nc.tensor.matmul(
                                psum,
                                lhsT=h8c[:, 2 * cc2:2 * cc2 + 2, ts(j, P)],
                                rhs=wv_s[:, 2 * cc2:2 * cc2 + 2, ts(half, 512)],
                                start=(cc2 == 0), stop=(cc2 == 3), perf_mode=DR)
                        vdst = Vr[:, st, half * 8:(half + 1) * 8, 0:64]
                        vsrc = psum.rearrange("p (h d) -> p h d", d=64)
                        if half == 0:
                            nc.vector.tensor_scalar_mul(
                                vdst, vsrc, gate_s[:, st:st + 1])
                        else:
                            nc.scalar.mul(vdst, vsrc, gate_s[:, st:st + 1])

                if ch == 0:
                    # Q for the own 512 tokens (= permuted positions 0..511);
                    # scale folds C^-0.5 and both 1/32 weight scales.
                    for pair in range(8):
                        psum = pp_a.tile([P, 512], f32, tag="mm")
                        for cc2 in range(4):
                            nc.tensor.matmul(
                                psum,
                                lhsT=wq_s[:, 2 * cc2:2 * cc2 + 2, pair, :],
                                rhs=h8c[:, 2 * cc2:2 * cc2 + 2, :],
                                start=(cc2 == 0), stop=(cc2 == 3), perf_mode=DR)
                        qsc = float(C) ** -0.5 / (WS * WS)
                        if pair % 2 == 0:
                            nc.vector.tensor_scalar_mul(QT[:, pair, :], psum, qsc)
                        else:
                            nc.scalar.mul(QT[:, pair, :], psum, qsc)

            # software pipeline: emit chunk ch+1's stats before chunk ch's
            # normalize+projections so the PE never stalls on the LN chain
            pend = None
            for ch in range(4):
                if ch == 0:
                    xc = xc0
                else:
                    xc = pxc.tile([P, CC, 512], bf, tag="xkv")
                    nc.sync.dma_start(xc, xkvT[:, :, ts(ch, 512)])
                a, ma = _ln_stats(nc, st_a, p1, p1t, xc, eps1, ones1)
                if pend is not None:
                    pch, pxt, pa, pma = pend
                    h8c = h8[:, :, ts(pch, 512)]
                    _ln_norm(nc, p1t, pxt, h8c, pa, pma)
                    chunk_projs(pch, h8c)
                pend = (ch, xc, a, ma)
            pch, pxt, pa, pma = pend
            h8c = h8[:, :, ts(pch, 512)]
            _ln_norm(nc, p1t, pxt, h8c, pa, pma)
            chunk_projs(pch, h8c)

        # prefetch for phase C while attention runs
        xq_sb = load(pf, xqT, (P, CC, TQ), f32, tag="xq_res")
        wo_s = load(pf, wo, (P, CC, 8, P), fp8)

        # ---- phase B: attention ----
        OT8 = opool.tile([P, 8, TQ], fp8)
        with tc.tile_pool(name="p_att", bufs=3) as pat, \
             tc.tile_pool(name="p_scr", bufs=2, space="PSUM") as pscr, \
             tc.tile_pool(name="p_av", bufs=2, space="PSUM") as pav:
            for h in range(H):
                pair, half = h // 2, h % 2
                hp = slice(64 * half, 64 * half + 64)
                avp = pav.tile([65, TQ], f32, tag="av")
                for sp in range(8):
                    scp = pscr.tile([P, 2, TQ], f32, tag="sc")
                    diag = sp < 2
                    for j in range(2):
                        nc.tensor.matmul(scp[:, j, :],
                                         lhsT=KT[hp, pair, ts(2 * sp + j, P)],
                                         rhs=QT[hp, pair, :],
                                         start=True, stop=not diag)
                        if diag:
                            nc.tensor.matmul(scp[:, j, :], lhsT=eye_s,
                                             rhs=mask_s[:, 2 * sp + j, :],
                                             start=False, stop=True)
                    e8 = pat.tile([P, 2, TQ], fp8, tag="e8")
                    nc.scalar.activation(
                        e8.rearrange("p a q -> p (a q)"),
                        scp.rearrange("p a q -> p (a q)"), AF.Exp)
                    nc.tensor.matmul(avp, lhsT=Vr[:, 2 * sp:2 * sp + 2, h, :],
                                     rhs=e8, start=(sp == 0), stop=(sp == 7),
                                     perf_mode=DR)
                zr = small.tile([1, TQ], f32, tag="zr")
                nc.vector.reciprocal(zr, avp[64:65, :])
                zb = pat.tile([64, TQ], f32, tag="zb")
                nc.gpsimd.partition_broadcast(zb, zr)
                nc.vector.scalar_tensor_tensor(
                    OT8[hp, pair, :], avp[0:64, :], OTS, zb,
                    op0=ALU.mult, op1=ALU.mult)
        kvq_ctx.close()

        # ---- phase C: output projection + residual; LN2 ----
        y1 = pf.tile([P, CC, TQ], f32)
        h28 = pf.tile([P, CC, TQ], fp8)
        with tc.tile_pool(name="p_ln2", bufs=1) as pl2, \
             tc.tile_pool(name="p_ln2t", bufs=2) as pl2t, \
             tc.tile_pool(name="pp_c", bufs=4, space="PSUM") as pp_c, \
             tc.tile_pool(name="st_c", bufs=1, space="PSUM") as st_c:
            y1b = pl2.tile([P, CC, TQ], bf)
            for mo in range(CC):
                psum = pp_c.tile([P, 512], f32, tag="mm")
                for cc2 in range(4):
                    nc.tensor.matmul(psum,
                                     lhsT=wo_s[:, 2 * cc2:2 * cc2 + 2, mo, :],
                                     rhs=OT8[:, 2 * cc2:2 * cc2 + 2, :],
                                     start=(cc2 == 0), stop=(cc2 == 3),
                                     perf_mode=DR)
                # y1 = psum/(WS*OTS) + (x + bo)  [bo folded into xqT on host]
                nc.vector.scalar_tensor_tensor(
                    y1[:, mo, :], psum, 1.0 / (WS * OTS), xq_sb[:, mo, :],
                    op0=ALU.mult, op1=ALU.add)
                nc.scalar.copy(y1b[:, mo, :], y1[:, mo, :])
            a2, ma2 = _ln_stats(nc, st_c, pl2, pl2t, y1b, eps1, ones1)
            _ln_norm(nc, pl2t, y1b, h28, a2, ma2, on_pool=0)

        # ---- phase D: FFN ----
        with tc.tile_pool(name="p_ffn", bufs=1) as pff, \
             tc.tile_pool(name="p_ffnt", bufs=2) as pft, \
             tc.tile_pool(name="p_wstream", bufs=4) as pws, \
             tc.tile_pool(name="p_w2s", bufs=8) as pw2, \
             tc.tile_pool(name="pp_d", bufs=4, space="PSUM") as pp_d:
            zT = pff.tile([P, 32, TQ], fp8)
            for m in range(32):
                w1b = pws.tile([P, CC, P], fp8, tag="w1")
                nc.sync.dma_start(w1b, w1[m])
                psum = pp_d.tile([P, 512], f32, tag="mm")
                for cc2 in range(4):
                    nc.tensor.matmul(psum,
                                     lhsT=w1b[:, 2 * cc2:2 * cc2 + 2, :],
                                     rhs=h28[:, 2 * cc2:2 * cc2 + 2, :],
                                     start=(cc2 == 0), stop=(cc2 == 3),
                                     perf_mode=DR)
                # z = relu(psum + 32*b1) carries x32; b1 pre-scaled on host.
                # Alternate Act/DVE so neither engine gates the FFN1 rate.
                if m % 2 == 0:
                    nc.scalar.activation(zT[:, m, :], psum, AF.Relu,
                                         bias=b1_s[:, m:m + 1])
                else:
                    nc.vector.tensor_scalar(zT[:, m, :], psum,
                                            scalar1=b1_s[:, m:m + 1],
                                            scalar2=0.0,
                                            op0=ALU.add, op1=ALU.max)

            for mo in range(CC):
                w2b = pw2.tile([P, 32, P], fp8, tag="w2")
                nc.sync.dma_start(w2b, w2[mo])
                psum = pp_d.tile([P, 512], f32, tag="mm")
                for ff2 in range(16):
                    nc.tensor.matmul(psum,
                                     lhsT=w2b[:, 2 * ff2:2 * ff2 + 2, :],
                                     rhs=zT[:, 2 * ff2:2 * ff2 + 2, :],
                                     start=(ff2 == 0), stop=(ff2 == 15),
                                     perf_mode=DR)
                t = pft.tile([P, TQ], f32, tag="res")
                nc.vector.tensor_scalar(t, psum, scalar1=1.0 / (WS * WS),
                                        scalar2=b2_s[:, mo:mo + 1],
                                        op0=ALU.mult, op1=ALU.add)
                ot = pft.tile([P, TQ], f32, tag="ot")
                nc.vector.tensor_tensor(ot, t, y1[:, mo, :], ALU.add)
                nc.sync.dma_start(outT[:, mo, :], ot)


_NC_CACHE = {}


def build_nc(reps=1):
    global _NC_CACHE
    key = reps
    if key in _NC_CACHE:
        return _NC_CACHE[key]
    nc = bacc.Bacc("TRN2", target_bir_lowering=False, debug=False,
                   enable_asserts=False, num_devices=8)

    def dram(name, shape, dtype, kind="ExternalInput"):
        return nc.dram_tensor(name, shape, dtype, kind=kind).ap()

    aps = (
        dram("xkvT", (P, CC, T), bf),
        dram("xqT", (P, CC, TQ), f32),
        dram("maskT", (P, NDIAG, TQ), bf),
        dram("gateT", (P, NSCH), f32),
        dram("vonesT", (P, NSCH, H), fp8),
        dram("eyeT", (P, P), bf),
        dram("wq", (P, CC, 8, P), fp8),
        dram("wk", (P, CC, 8, P), fp8),
        dram("wv", (P, CC, C), fp8),
        dram("wo", (P, CC, 8, P), fp8),
        dram("w1", (32, P, CC, P), fp8),
        dram("w2", (CC, P, 32, P), fp8),
        dram("b1_t", (P, 32), f32),
        dram("b2_t", (P, CC), f32),
        dram("outT", (P, CC, TQ), f32, kind="ExternalOutput"),
    )
    with tile.TileContext(nc) as tc:
        for _ in range(reps):
            _body(nc, tc, aps)
    nc.compile()
    _NC_CACHE[key] = nc
    return nc


def _tile_lhst(w, dt=f8):  # (C_in, C_out) -> (P, cc, pair/mo, 128)
    return np.ascontiguousarray(
        w.reshape(CC, P, 8, P).transpose(1, 0, 2, 3)).astype(dt)


def make_in_maps(inputs):
    """Build the 8 per-core input dicts from the full problem inputs."""
    x = np.asarray(inputs["x"], np.float32)
    Wq = np.asarray(inputs["Wq"], np.float32)
    Wk = np.asarray(inputs["Wk"], np.float32)
    Wv = np.asarray(inputs["Wv"], np.float32)
    Wo = np.asarray(inputs["Wo"], np.float32)
    bo = np.asarray(inputs["bo"], np.float32)
    W1 = np.asarray(inputs["W1"], np.float32)
    b1 = np.asarray(inputs["b1"], np.float32)
    W2 = np.asarray(inputs["W2"], np.float32)
    b2 = np.asarray(inputs["b2"], np.float32)
    g1 = np.asarray(inputs["g1"], np.float32)
    be1 = np.asarray(inputs["be1"], np.float32)
    g2 = np.asarray(inputs["g2"], np.float32)
    be2 = np.asarray(inputs["be2"], np.float32)
    assert np.all(be1 == 0.0), "be1 folding not implemented"

    # fold LN affine into weights: (h*g + be) @ W
    wq_f = (Wq * g1[None, :, None]).transpose(1, 0, 2).reshape(C, C)
    wk_f = (Wk * g1[None, :, None]).transpose(1, 0, 2).reshape(C, C)
    wv_f = (Wv * g1[None, :, None]).transpose(1, 0, 2).reshape(C, C)
    w1_f = W1 * g2[:, None]
    b1_f = b1 + be2 @ W1  # be2 folds into the FFN bias

    shared = {
        "wq": _tile_lhst(wq_f * WS),
        "wk": _tile_lhst(wk_f * WS),
        "wv": np.ascontiguousarray(
            (wv_f * WS).reshape(CC, P, C).transpose(1, 0, 2)).astype(f8),
        "wo": _tile_lhst(Wo * WS),
        "w1": np.ascontiguousarray(
            (w1_f * WS).reshape(CC, P, 32, P).transpose(2, 1, 0, 3)).astype(f8),
        "w2": np.ascontiguousarray(
            (W2 * WS).reshape(32, P, CC, P).transpose(2, 1, 0, 3)).astype(f8),
        "b1_t": np.ascontiguousarray((b1_f * WS).reshape(32, P).T),
        "b2_t": np.ascontiguousarray(b2.reshape(CC, P).T),
        "eyeT": np.eye(P, dtype=bf16),
    }

    # core-invariant diagonal mask: key position 128*c+i vs query j
    i = np.arange(P)
    j = np.arange(TQ)
    maskT = np.stack(
        [np.where((c * P + i)[:, None] > j[None, :], np.float32(NEG), 0.0)
         for c in range(NDIAG)], axis=1).astype(bf16)   # (P, 4, TQ)
    shared["maskT"] = np.ascontiguousarray(maskT)

    in_maps = []
    for core in range(8):
        b, a = core // 4, core % 4
        q0 = TQ * a
        perm = np.concatenate([np.arange(q0, q0 + TQ), np.arange(0, q0),
                               np.arange(q0 + TQ, T)])
        xbT = np.ascontiguousarray(x[b][perm].T)             # (C, T) permuted
        xkvT = xbT.reshape(CC, P, T).transpose(1, 0, 2).astype(bf16)
        xq = x[b][q0:q0 + TQ] + bo[None, :]                  # fold bo
        xqT = np.ascontiguousarray(xq.T.reshape(CC, P, TQ).transpose(1, 0, 2))

        gate01 = np.zeros(NSCH, np.float32)
        gate01[:NDIAG] = 1.0
        gate01[NDIAG:NDIAG + 4 * a] = 1.0
        m = {
            "xkvT": np.ascontiguousarray(xkvT),
            "xqT": xqT.astype(np.float32),
            "gateT": np.ascontiguousarray(
                np.broadcast_to(gate01 / WS, (P, NSCH))).astype(np.float32),
            "vonesT": np.ascontiguousarray(np.broadcast_to(
                gate01[None, :, None], (P, NSCH, H))).astype(f8),
            **shared,
        }
        in_maps.append(m)
    return in_maps


def assemble_output(core_outs):
    """core_outs: list of 8 dicts with 'outT' (P, CC, TQ) fp32."""
    out = np.zeros((B, T, C), np.float32)
    for c in range(8):
        b, a = c // 4, c % 4
        y2 = core_outs[c]["outT"].transpose(1, 0, 2).reshape(C, TQ)
        out[b, TQ * a:TQ * (a + 1), :] = y2.T
    return out


def kernel(**inputs) -> np.ndarray:
    nc = build_nc()
    in_maps = make_in_maps(inputs)
    res = bass_utils.run_bass_kernel_spmd(nc, in_maps, core_ids=list(range(8)))
    return assemble_output(res.results)


if __name__ == "__main__":
    import reference
    inputs = {k: np.asarray(v) for k, v in reference.setup_inputs().items()}
    expected = np.asarray(reference.reference(**inputs))
    actual = kernel(**inputs)
    err = np.abs(actual - expected)
    print("absmax err:", err.max(), "scale:", np.abs(expected).max())
    print("rel fro:", np.linalg.norm(actual - expected) / np.linalg.norm(expected))


# revision 35
# speedup vs baseline: 1.0141x; 1.0141x over previous
"""Trainium2 Bass kernel for a dense transformer block (LN -> 16-head causal
attention -> residual -> LN -> FFN -> residual) on x:(2, 2048, 1024) fp32.

Sharding: 8 cores, zero collectives. Core c handles batch b=c//4, query chunk
a=c%4 (512 contiguous tokens). Every core recomputes full-sequence K/V for its
batch (replicated compute instead of collectives: the cost model prices an
AllGather of K/V at ~226us, far above the ~28us of redundant projection work).

Key tricks (all data-driven so the compiled program is identical across cores):
- Token permutation: each core's K/V token order puts its OWN 512 query tokens
  first. The 4 leading 128-token key chunks are then exactly the "diagonal"
  causal blocks for every core, so the additive causal mask is a core-invariant
  constant applied to a fixed set of psum blocks (folded into the score matmul
  via an identity-lhsT accumulate, not a vector op).
- V gating: chunks that a core's queries may never attend (future tokens) are
  zeroed at V-evacuation time via a per-core gate column, and the softmax
  denominator "ones" column is gated the same way. exp() of those scores still
  runs (uniform program) but contributes exactly 0.
- fp8 (e4m3) DoubleRow matmuls for Q/K/V/out projections, AV, and both FFN
  layers: weights are pre-scaled by 32 on the host to sit in e4m3's sweet spot;
  the inverse scales fold into existing psum-evacuation ops. Scores stay bf16
  (contraction is 64-deep; DoubleRow needs 128-pairs). LayerNorm scale/shift
  (g, be) fold into the weights/biases on the host.
- LayerNorm statistics via ones-vector matmuls (partition reduction); the four
  512-token chunks' stats land on psum (partition 0/64 x free half) so the
  scalar math runs once over (65, 2, 512) views.
- softmax exp runs on the Activation engine over PAIRS of score psum banks
  (one instruction per 2 key-chunks), writing fp8 pairs consumed directly by
  DoubleRow AV matmuls.
Engine balance: psum evacuations are spread over DVE/Act/Pool (K-evac + square
on Act, normalize-subtract on Pool, rest on DVE).
"""

import numpy as np
import ml_dtypes

import concourse.bass as bass
import concourse.tile as tile
from concourse import bacc, mybir
from concourse import bass_utils
from concourse.bass import ts

P = 128
B, T, C = 2, 2048, 1024
H, D = 16, 64
FF = 4 * C
CC = C // P          # 8 feature chunks
TQ = 512             # queries per core
NSCH = T // P        # 16 key chunks
NDIAG = 4            # leading chunks = the core's own (diagonal) tokens
EPS = 1e-5
NEG = -30000.0
WS = 32.0            # host-side fp8 weight scale
OTS = 64.0           # attention-output fp8 scale

bf16 = ml_dtypes.bfloat16
f8 = ml_dtypes.float8_e4m3fn

f32 = mybir.dt.float32
bf = mybir.dt.bfloat16
fp8 = mybir.dt.float8e4
AF = mybir.ActivationFunctionType
ALU = mybir.AluOpType
DR = mybir.MatmulPerfMode.DoubleRow


def _ln_stats(nc, stp, big1, chunked, xs, eps1, ones1):
    """LN stats of one 512-token chunk xs (P, CC, 512) bf16. Sum lands on
    psum partition 0, sum-of-squares on partition 64 (one bank). Returns
    (a, ma) (1, 512) bf16 tiles: rsqrt(var) and mean*rsqrt(var)."""
    st = stp.tile([65, 512], f32, tag="st")
    ps = st[0:1, :]
    pq = st[64:65, :]
    for cc in range(CC):
        nc.tensor.matmul(ps, lhsT=ones1, rhs=xs[:, cc, :],
                         start=(cc == 0), stop=(cc == CC - 1))
    for cc in range(CC):
        sq = chunked.tile([P, 512], bf, tag="st_sqt")
        nc.scalar.activation(sq, xs[:, cc, :], AF.Square)
        nc.tensor.matmul(pq, lhsT=ones1, rhs=sq,
                         start=(cc == 0), stop=(cc == CC - 1))
    nc.vector.tensor_scalar_mul(st, st, 1.0 / C)   # m, E[x^2] (garbage rows ok)
    m, v = ps, pq
    msq = big1.tile([1, 512], f32, tag="st_tmp")
    nc.scalar.activation(msq, m, AF.Square)
    nc.vector.tensor_tensor(v, v, msq, ALU.subtract)      # v := var
    sd = big1.tile([1, 512], f32, tag="st_tmp2")
    nc.scalar.activation(sd, v, AF.Sqrt, bias=eps1[0:1, :])
    a = big1.tile([1, 512], bf, tag="st_a")
    with nc.allow_low_precision(reason="LN scale in bf16 is within budget"):
        nc.vector.reciprocal(a, sd)
    ma = big1.tile([1, 512], bf, tag="st_ma")
    nc.vector.tensor_mul(ma, m, a)
    return a, ma


def _ln_norm(nc, chunked, xs, out8c, a, ma, on_pool=3):
    """out8c fp8 = (xs - m) * a. mult on DVE; subtract split DVE/Pool
    (on_pool = out of 8 cc on Pool)."""
    a_bc = chunked.tile([P, 512], bf, tag="nm_abc")
    nc.gpsimd.partition_broadcast(a_bc, a)
    ma_bc = chunked.tile([P, 512], bf, tag="nm_mabc")
    nc.gpsimd.partition_broadcast(ma_bc, ma)
    for cc in range(CC):
        t = chunked.tile([P, 512], bf, tag="nm_t")
        nc.vector.tensor_mul(t, xs[:, cc, :], a_bc)
        if cc % 2 == 0 and on_pool:
            nc.gpsimd.tensor_tensor(out8c[:, cc, :], t, ma_bc, ALU.subtract)
        else:
            nc.vector.tensor_tensor(out8c[:, cc, :], t, ma_bc, ALU.subtract)


def _body(nc, tc, aps):
    (xkvT, xqT, maskT, gateT, vonesT, eyeT, wq, wk, wv, wo, w1, w2,
     b1_t, b2_t, outT) = aps

    import contextlib
    ctx = contextlib.ExitStack()
    with ctx:
        consts = ctx.enter_context(tc.tile_pool(name="consts", bufs=1))
        small = ctx.enter_context(tc.tile_pool(name="small", bufs=2))

        ones1 = consts.tile([P, 1], bf)
        nc.vector.memset(ones1, 1.0)
        eps1 = consts.tile([P, 1], f32)
        nc.vector.memset(eps1, EPS)

        def load(pool, ap_dram, shape, dtype=f32, tag=None):
            t = pool.tile(list(shape), dtype, tag=tag or ap_dram.name)
            nc.sync.dma_start(t, ap_dram)
            return t

        eye_s = load(consts, eyeT, (P, P), bf)
        gate_s = load(consts, gateT, (P, NSCH))
        b1_s = load(consts, b1_t, (P, 32))
        b2_s = load(consts, b2_t, (P, CC))
        mask_s = load(consts, maskT, (P, NDIAG, TQ), bf)
        vones_s = load(consts, vonesT, (P, NSCH, H), fp8)

        opool = ctx.enter_context(tc.tile_pool(name="opool", bufs=1))
        pf = ctx.enter_context(tc.tile_pool(name="p_late", bufs=1))

        # ---- long-lived activation tiles
        kvq_ctx = contextlib.ExitStack()
        kvq = kvq_ctx.enter_context(tc.tile_pool(name="kvq", bufs=1))
        KT = kvq.tile([P, 8, T], bf)
        Vr = kvq.tile([P, NSCH, H, 65], fp8)
        QT = kvq.tile([P, 8, TQ], bf)

        # ---- phase A: LN1 + Q/K/V projections, pipelined per 512-chunk ----
        with tc.tile_pool(name="hpool", bufs=1) as hpool, \
             tc.tile_pool(name="p_xc", bufs=2) as pxc, \
             tc.tile_pool(name="p_ln1", bufs=2) as p1, \
             tc.tile_pool(name="p_ln1t", bufs=3) as p1t, \
             tc.tile_pool(name="p_w", bufs=1) as pw, \
             tc.tile_pool(name="pp_a", bufs=4, space="PSUM") as pp_a, \
             tc.tile_pool(name="st_a", bufs=2, space="PSUM") as st_a:
            xc0 = pxc.tile([P, CC, 512], bf, tag="xkv")
            nc.sync.dma_start(xc0, xkvT[:, :, ts(0, 512)])
            wk_s = load(pw, wk, (P, CC, 8, P), fp8)
            wv_s = load(pw, wv, (P, CC, C), fp8)
            wq_s = load(pw, wq, (P, CC, 8, P), fp8)
            h8 = hpool.tile([P, CC, T], fp8)
            # gated ones column for the softmax denominator
            nc.vector.tensor_copy(
                Vr[:, :, :, 64:65].rearrange("p a h e -> p a (h e)"), vones_s)

            def chunk_projs(ch, h8c):
                # K rows for this chunk's 512 tokens (all 8 feature pairs).
                # KT carries the x32 weight scale (folded into Q's evac).
                for pair in range(8):
                    psum = pp_a.tile([P, 512], f32, tag="mm")
                    for cc2 in range(4):
                        nc.tensor.matmul(
                            psum,
                            lhsT=wk_s[:, 2 * cc2:2 * cc2 + 2, pair, :],
                            rhs=h8c[:, 2 * cc2:2 * cc2 + 2, :],
                            start=(cc2 == 0), stop=(cc2 == 3), perf_mode=DR)
                    nc.scalar.copy(KT[:, pair, ts(ch, 512)], psum)

                # V rows (4 token subchunks x 2 feature halves); gated evac
                # (gate carries the 1/32 rescale), fp8 out, split DVE/Act.
                for j in range(4):
                    st = 4 * ch + j
                    for half in range(2):
                        psum = pp_a.tile([P, 512], f32, tag="mm")
                        for cc2 in range(4):
                            nc.tensor.matmul(
                                psum,
                                lhsT=h8c[:, 2 * cc2:2 * cc2 + 2, ts(j, P)],
                                rhs=wv_s[:, 2 * cc2:2 * cc2 + 2, ts(half, 512)],
                                start=(cc2 == 0), stop=(cc2 == 3), perf_mode=DR)
                        vdst = Vr[:, st, half * 8:(half + 1) * 8, 0:64]
                        vsrc = psum.rearrange("p (h d) -> p h d", d=64)
                        if half == 0:
                            nc.vector.tensor_scalar_mul(
                                vdst, vsrc, gate_s[:, st:st + 1])
                        else:
                            nc.scalar.mul(vdst, vsrc, gate_s[:, st:st + 1])

                if ch == 0:
                    # Q for the own 512 tokens (= permuted positions 0..511);
                    # scale folds C^-0.5 and both 1/32 weight scales.
                    for pair in range(8):
                        psum = pp_a.tile([P, 512], f32, tag="mm")
                        for cc2 in range(4):
                            nc.tensor.matmul(
                                psum,
                                lhsT=wq_s[:, 2 * cc2:2 * cc2 + 2, pair, :],
                                rhs=h8c[:, 2 * cc2:2 * cc2 + 2, :],
                                start=(cc2 == 0), stop=(cc2 == 3), perf_mode=DR)
                        qsc = float(C) ** -0.5 / (WS * WS)
                        if pair % 2 == 0:
                            nc.vector.tensor_scalar_mul(QT[:, pair, :], psum, qsc)
                        else:
                            nc.scalar.mul(QT[:, pair, :], psum, qsc)

            # software pipeline: emit chunk ch+1's stats before chunk ch's
            # normalize+projections so the PE never stalls on the LN chain
            pend = None
            for ch in range(4):
                if ch == 0:
                    xc = xc0
                else:
                    xc = pxc.tile([P, CC, 512], bf, tag="xkv")
                    nc.sync.dma_start(xc, xkvT[:, :, ts(ch, 512)])
                a, ma = _ln_stats(nc, st_a, p1, p1t, xc, eps1, ones1)
                if pend is not None:
                    pch, pxt, pa, pma = pend
                    h8c = h8[:, :, ts(pch, 512)]
                    _ln_norm(nc, p1t, pxt, h8c, pa, pma)
                    chunk_projs(pch, h8c)
                pend = (ch, xc, a, ma)
            pch, pxt, pa, pma = pend
            h8c = h8[:, :, ts(pch, 512)]
            _ln_norm(nc, p1t, pxt, h8c, pa, pma)
            chunk_projs(pch, h8c)

        # prefetch for phase C while attention runs
        xq_sb = load(pf, xqT, (P, CC, TQ), f32, tag="xq_res")
        wo_s = load(pf, wo, (P, CC, 8, P), fp8)

        # ---- phase B: attention ----
        OT8 = opool.tile([P, 8, TQ], fp8)
        with tc.tile_pool(name="p_att", bufs=3) as pat, \
             tc.tile_pool(name="p_scr", bufs=3, space="PSUM") as pscr, \
             tc.tile_pool(name="p_av", bufs=2, space="PSUM") as pav:
            for h in range(H):
                pair, half = h // 2, h % 2
                hp = slice(64 * half, 64 * half + 64)
                avp = pav.tile([65, TQ], f32, tag="av")
                for sp in range(8):
                    scp = pscr.tile([P, 2, TQ], f32, tag="sc")
                    diag = sp < 2
                    for j in range(2):
                        nc.tensor.matmul(scp[:, j, :],
                                         lhsT=KT[hp, pair, ts(2 * sp + j, P)],
                                         rhs=QT[hp, pair, :],
                                         start=True, stop=not diag)
                        if diag:
                            nc.tensor.matmul(scp[:, j, :], lhsT=eye_s,
                                             rhs=mask_s[:, 2 * sp + j, :],
                                             start=False, stop=True)
                    e8 = pat.tile([P, 2, TQ], fp8, tag="e8")
                    nc.scalar.activation(
                        e8.rearrange("p a q -> p (a q)"),
                        scp.rearrange("p a q -> p (a q)"), AF.Exp)
                    nc.tensor.matmul(avp, lhsT=Vr[:, 2 * sp:2 * sp + 2, h, :],
                                     rhs=e8, start=(sp == 0), stop=(sp == 7),
                                     perf_mode=DR)
                zr = small.tile([1, TQ], f32, tag="zr")
                nc.vector.reciprocal(zr, avp[64:65, :])
                zb = pat.tile([64, TQ], f32, tag="zb")
                nc.gpsimd.partition_broadcast(zb, zr)
                nc.vector.scalar_tensor_tensor(
                    OT8[hp, pair, :], avp[0:64, :], OTS, zb,
                    op0=ALU.mult, op1=ALU.mult)
        kvq_ctx.close()

        # ---- phase C: output projection + residual; LN2 ----
        y1 = pf.tile([P, CC, TQ], f32)
        h28 = pf.tile([P, CC, TQ], fp8)
        with tc.tile_pool(name="p_ln2", bufs=1) as pl2, \
             tc.tile_pool(name="p_ln2t", bufs=2) as pl2t, \
             tc.tile_pool(name="pp_c", bufs=4, space="PSUM") as pp_c, \
             tc.tile_pool(name="st_c", bufs=1, space="PSUM") as st_c:
            y1b = pl2.tile([P, CC, TQ], bf)
            for mo in range(CC):
                psum = pp_c.tile([P, 512], f32, tag="mm")
                for cc2 in range(4):
                    nc.tensor.matmul(psum,
                                     lhsT=wo_s[:, 2 * cc2:2 * cc2 + 2, mo, :],
                                     rhs=OT8[:, 2 * cc2:2 * cc2 + 2, :],
                                     start=(cc2 == 0), stop=(cc2 == 3),
                                     perf_mode=DR)
                # y1 = psum/(WS*OTS) + (x + bo)  [bo folded into xqT on host]
                nc.vector.scalar_tensor_tensor(
                    y1[:, mo, :], psum, 1.0 / (WS * OTS), xq_sb[:, mo, :],
                    op0=ALU.mult, op1=ALU.add)
                nc.scalar.copy(y1b[:, mo, :], y1[:, mo, :])
            a2, ma2 = _ln_stats(nc, st_c, pl2, pl2t, y1b, eps1, ones1)
            _ln_norm(nc, pl2t, y1b, h28, a2, ma2, on_pool=0)

        # ---- phase D: FFN ----
        with tc.tile_pool(name="p_ffn", bufs=1) as pff, \
             tc.tile_pool(name="p_ffnt", bufs=2) as pft, \
             tc.tile_pool(name="p_wstream", bufs=4) as pws, \
             tc.tile_pool(name="p_w2s", bufs=8) as pw2, \
             tc.tile_pool(name="pp_d", bufs=4, space="PSUM") as pp_d:
            zT = pff.tile([P, 32, TQ], fp8)
            for m in range(32):
                w1b = pws.tile([P, CC, P], fp8, tag="w1")
                nc.sync.dma_start(w1b, w1[m])
                psum = pp_d.tile([P, 512], f32, tag="mm")
                for cc2 in range(4):
                    nc.tensor.matmul(psum,
                                     lhsT=w1b[:, 2 * cc2:2 * cc2 + 2, :],
                                     rhs=h28[:, 2 * cc2:2 * cc2 + 2, :],
                                     start=(cc2 == 0), stop=(cc2 == 3),
                                     perf_mode=DR)
                # z = relu(psum + 32*b1) carries x32; b1 pre-scaled on host.
                # Alternate Act/DVE so neither engine gates the FFN1 rate.
                if m % 2 == 0:
                    nc.scalar.activation(zT[:, m, :], psum, AF.Relu,
                                         bias=b1_s[:, m:m + 1])
                else:
                    nc.vector.tensor_scalar(zT[:, m, :], psum,
                                            scalar1=b1_s[:, m:m + 1],
                                            scalar2=0.0,
                                            op0=ALU.add, op1=ALU.max)

            for mo in range(CC):
                w2b = pw2.tile([P, 32, P], fp8, tag="w2")
                nc.sync.dma_start(w2b, w2[mo])
                psum = pp_d.tile([P, 512], f32, tag="mm")
                for ff2 in range(16):
                    nc.tensor.matmul(psum,
                                     lhsT=w2b[:, 2 * ff2:2 * ff2 + 2, :],
                                     rhs=zT[:, 2 * ff2:2 * ff2 + 2, :],
                                     start=(ff2 == 0), stop=(ff2 == 15),
                                     perf_mode=DR)
                t = pft.tile([P, TQ], f32, tag="res")
                nc.vector.tensor_scalar(t, psum, scalar1=1.0 / (WS * WS),
                                        scalar2=b2_s[:, mo:mo + 1],
                                        op0=ALU.mult, op1=ALU.add)
                ot = pft.tile([P, TQ], f32, tag="ot")
                nc.vector.tensor_tensor(ot, t, y1[:, mo, :], ALU.add)
                nc.sync.dma_start(outT[:, mo, :], ot)


_NC_CACHE = {}


def build_nc(reps=1):
    global _NC_CACHE
    key = reps
    if key in _NC_CACHE:
        return _NC_CACHE[key]
    nc = bacc.Bacc("TRN2", target_bir_lowering=False, debug=False,
                   enable_asserts=False, num_devices=8)

    def dram(name, shape, dtype, kind="ExternalInput"):
        return nc.dram_tensor(name, shape, dtype, kind=kind).ap()

    aps = (
        dram("xkvT", (P, CC, T), bf),
        dram("xqT", (P, CC, TQ), f32),
        dram("maskT", (P, NDIAG, TQ), bf),
        dram("gateT", (P, NSCH), f32),
        dram("vonesT", (P, NSCH, H), fp8),
        dram("eyeT", (P, P), bf),
        dram("wq", (P, CC, 8, P), fp8),
        dram("wk", (P, CC, 8, P), fp8),
        dram("wv", (P, CC, C), fp8),
        dram("wo", (P, CC, 8, P), fp8),
        dram("w1", (32, P, CC, P), fp8),
        dram("w2", (CC, P, 32, P), fp8),
        dram("b1_t", (P, 32), f32),
        dram("b2_t", (P, CC), f32),
        dram("outT", (P, CC, TQ), f32, kind="ExternalOutput"),
    )
    with tile.TileContext(nc) as tc:
        for _ in range(reps):
            _body(nc, tc, aps)
    nc.compile()
    _NC_CACHE[key] = nc
    return nc


def _tile_lhst(w, dt=f8):  # (C_in, C_out) -> (P, cc, pair/mo, 128)
    return np.ascontiguousarray(
        w.reshape(CC, P, 8, P).transpose(1, 0, 2, 3)).astype(dt)


def make_in_maps(inputs):
    """Build the 8 per-core input dicts from the full problem inputs."""
    x = np.asarray(inputs["x"], np.float32)
    Wq = np.asarray(inputs["Wq"], np.float32)
    Wk = np.asarray(inputs["Wk"], np.float32)
    Wv = np.asarray(inputs["Wv"], np.float32)
    Wo = np.asarray(inputs["Wo"], np.float32)
    bo = np.asarray(inputs["bo"], np.float32)
    W1 = np.asarray(inputs["W1"], np.float32)
    b1 = np.asarray(inputs["b1"], np.float32)
    W2 = np.asarray(inputs["W2"], np.float32)
    b2 = np.asarray(inputs["b2"], np.float32)
    g1 = np.asarray(inputs["g1"], np.float32)
    be1 = np.asarray(inputs["be1"], np.float32)
    g2 = np.asarray(inputs["g2"], np.float32)
    be2 = np.asarray(inputs["be2"], np.float32)
    assert np.all(be1 == 0.0), "be1 folding not implemented"

    # fold LN affine into weights: (h*g + be) @ W
    wq_f = (Wq * g1[None, :, None]).transpose(1, 0, 2).reshape(C, C)
    wk_f = (Wk * g1[None, :, None]).transpose(1, 0, 2).reshape(C, C)
    wv_f = (Wv * g1[None, :, None]).transpose(1, 0, 2).reshape(C, C)
    w1_f = W1 * g2[:, None]
    b1_f = b1 + be2 @ W1  # be2 folds into the FFN bias

    shared = {
        "wq": _tile_lhst(wq_f * WS),
        "wk": _tile_lhst(wk_f * WS),
        "wv": np.ascontiguousarray(
            (wv_f * WS).reshape(CC, P, C).transpose(1, 0, 2)).astype(f8),
        "wo": _tile_lhst(Wo * WS),
        "w1": np.ascontiguousarray(
            (w1_f * WS).reshape(CC, P, 32, P).transpose(2, 1, 0, 3)).astype(f8),
        "w2": np.ascontiguousarray(
            (W2 * WS).reshape(32, P, CC, P).transpose(2, 1, 0, 3)).astype(f8),
        "b1_t": np.ascontiguousarray((b1_f * WS).reshape(32, P).T),
        "b2_t": np.ascontiguousarray(b2.reshape(CC, P).T),
        "eyeT": np.eye(P, dtype=bf16),
    }

    # core-invariant diagonal mask: key position 128*c+i vs query j
    i = np.arange(P)
    j = np.arange(TQ)
    maskT = np.stack(
        [np.where((c * P + i)[:, None] > j[None, :], np.float32(NEG), 0.0)
         for c in range(NDIAG)], axis=1).astype(bf16)   # (P, 4, TQ)
    shared["maskT"] = np.ascontiguousarray(maskT)

    in_maps = []
    for core in range(8):
        b, a = core // 4, core % 4
        q0 = TQ * a
        perm = np.concatenate([np.arange(q0, q0 + TQ), np.arange(0, q0),
                               np.arange(q0 + TQ, T)])
        xbT = np.ascontiguousarray(x[b][perm].T)             # (C, T) permuted
        xkvT = xbT.reshape(CC, P, T).transpose(1, 0, 2).astype(bf16)
        xq = x[b][q0:q0 + TQ] + bo[None, :]                  # fold bo
        xqT = np.ascontiguousarray(xq.T.reshape(CC, P, TQ).transpose(1, 0, 2))

        gate01 = np.zeros(NSCH, np.float32)
        gate01[:NDIAG] = 1.0
        gate01[NDIAG:NDIAG + 4 * a] = 1.0
        m = {
            "xkvT": np.ascontiguousarray(xkvT),
            "xqT": xqT.astype(np.float32),
            "gateT": np.ascontiguousarray(
                np.broadcast_to(gate01 / WS, (P, NSCH))).astype(np.float32),
            "vonesT": np.ascontiguousarray(np.broadcast_to(
                gate01[None, :, None], (P, NSCH, H))).astype(f8),
            **shared,
        }
        in_maps.append(m)
    return in_maps


def assemble_output(core_outs):
    """core_outs: list of 8 dicts with 'outT' (P, CC, TQ) fp32."""
    out = np.zeros((B, T, C), np.float32)
    for c in range(8):
        b, a = c // 4, c % 4
        y2 = core_outs[c]["outT"].transpose(1, 0, 2).reshape(C, TQ)
        out[b, TQ * a:TQ * (a + 1), :] = y2.T
    return out


def kernel(**inputs) -> np.ndarray:
    nc = build_nc()
    in_maps = make_in_maps(inputs)
    res = bass_utils.run_bass_kernel_spmd(nc, in_maps, core_ids=list(range(8)))
    return assemble_output(res.results)


if __name__ == "__main__":
    import reference
    inputs = {k: np.asarray(v) for k, v in reference.setup_inputs().items()}
    expected = np.asarray(reference.reference(**inputs))
    actual = kernel(**inputs)
    err = np.abs(actual - expected)
    print("absmax err:", err.max(), "scale:", np.abs(expected).max())
    print("rel fro:", np.linalg.norm(actual - expected) / np.linalg.norm(expected))
